# revision 66
# baseline (speedup 1.0000x reference)
"""Trainium2 Bass kernel for channel-attention:
    scores[b,q,k] = sum_{h,w} Q[b,h,w,q] * K[b,h,w,k]
    attn = softmax_k(scores)
    out[b,h,w,q] = sum_k attn[b,q,k] * V[b,h,w,k]

Full inputs are [16, 128, 128, 64] f32. Data-parallel over batch across
8 NeuronCores (2 batches per core); no cross-core communication.

All data ships as fp16 (e5m10). fp16's 2^-11 rounding keeps the score
error ~0.003 absolute (scores have std ~128 over the 16384-term
contraction, fp32 PSUM accumulation), small enough that the softmax
output error stays ~4e-3 relative - no hi/lo split needed. This is the
key traffic win over the previous bf16 hi+lo kernel: 16 MiB per core
(q 4 + k 4 + vt 4 + out 4) instead of 24 MiB, with the kernel firmly
DMA-bound (the profile shows DMA active ~99% of the span).

Layouts (host-side prep on the unsharded numpy inputs):
  q16/k16: [B, H, W, C] fp16 - a w-pair slice [:, 2j:2j+2, :] is one
           contiguous [128, 128] matmul operand.
  vt:      [B, (dw c)=128, pair=W/2, H] fp16 - V transposed per w-pair
           so each pair is a ready [128, 128] lhsT tile.

Per-core dataflow (per batch):
  Phase A: per w-pair one LDWEIGHTS (q pair) + one N=128 matmul
    (k pair) accumulates into a [2C, 2, C] PSUM tile; the diagonal
    blocks hold the even-w and odd-w partial scores:
      scores = blocks[0:C, 0, :] + blocks[C:2C, 1, :]  (copy + add).
  Softmax over k (free dim): -max (DVE), exp with bias + accumulated
    row-sum (ACT), reciprocal + scale (DVE). attn^T via PE transpose,
    written twice into a block-diagonal [128, 128] fp16 tile (bd).
  Phase C: per w-pair one N=128 matmul (lhsT = V^T pair, rhs = bd)
    produces both output columns [h, (w0 q | w1 q)] in PSUM; 4 pairs per
    PSUM bank, copied to the fp16 out tile by DVE/ACT alternately.

Schedule (arrived at via perfetto-trace iteration; the load stream runs
~420 GB/s, the write path is slower and each HWDGE ring only keeps ~4
DMAs in flight, with consecutive DMAs on a ring separated by a
completion-receipt round trip):
  - ALL loads are emitted before any compute op, in consumption order
    q0k0, q1k1, vt0, vt1, pieces interleaved across the two HWDGE rings
    (sync + scalar) so each batch's q and k arrive together. No DMA
    ever queues behind a compute-gated instruction, and no store byte
    is ever injected into the (critical) load stream.
  - Compute order PA0, sm0, PA1, sm1, PC0, PC1: both score matrices
    finish as their bytes land; phase C starts the moment vt0 lands and
    the PE runs PC0+PC1 back-to-back to the end.
  - Stores all on the gpsimd/SWDGE ring in the natural [b, h, w, c]
    layout: batch 0 as two 1 MiB halves (separate otiles, so the
    first store issues the moment half the batch is copied - SWDGE
    first-byte latency is ~6 us from issue, making the first store's
    data readiness set the store-stream start), batch 1 as one 2 MiB
    store whose data lands while the earlier stores drain. Store
    bytes never interleave with the load stream on this fixture
    (reads drain first, then writes) and store transfers serialize
    globally; the [1, 1, 2] MiB split measured best ([0.5, 1.5, 2]
    and [2, 2] are both slower).

Measured on the 8-core axon fixture: ~59.5-60 us normal-phase, best
58.4 (fixture shows bursty contention phases adding up to ~8 us) vs
85 us for the previous bf16 hi+lo kernel, rel err 3.4e-3 (gate 2e-2).
"""

import sys

sys.path.insert(0, "/opt/trn_rl_repo")

import numpy as np

_B, _H, _W, _C = 16, 128, 128, 64
_NCORES = 8
_BPC = _B // _NCORES  # batches per core

_PIECE = 64  # w-columns per q/k load piece (1 MiB fp16)
_NP = _W // _PIECE
_PAIRS_TOT = _W // 2  # w-pairs per batch
_VPIECE = 32  # pairs per vt load piece (1 MiB)
_NVP = _PAIRS_TOT // _VPIECE

_cache = {}


def _build_nc():
    from contextlib import ExitStack

    import concourse.bass as bass  # noqa: F401
    import concourse.tile as tile
    from concourse import bacc, mybir
    from concourse.masks import make_identity

    f32 = mybir.dt.float32
    fp16 = mybir.dt.float16
    nc = bacc.Bacc(target_bir_lowering=False)

    q_ext = nc.declare_dram_parameter("q16", [_BPC, _H, _W, _C], fp16, isOutput=False)
    k_ext = nc.declare_dram_parameter("k16", [_BPC, _H, _W, _C], fp16, isOutput=False)
    vt_ext = nc.declare_dram_parameter(
        "vt", [_BPC, 2 * _C, _PAIRS_TOT, _H], fp16, isOutput=False
    )
    # one whole-batch 2 MiB store per batch: store byte-streams
    # serialize globally (one at a time at ~380 GB/s with ~1.2 us
    # turnaround between transfers), so the fewest stores win; the
    # natural [b, h, w, c] layout is fully contiguous per batch
    o_ext = nc.declare_dram_parameter("out", [_BPC, _H, _W, _C], fp16, isOutput=True)

    with tile.TileContext(nc) as tc, ExitStack() as ctx:
        singles = ctx.enter_context(tc.tile_pool(name="singles", bufs=1))
        qp = ctx.enter_context(tc.tile_pool(name="qp", bufs=2 * _NP))
        kp = ctx.enter_context(tc.tile_pool(name="kp", bufs=2 * _NP))
        vtp = ctx.enter_context(tc.tile_pool(name="vtp", bufs=2 * _NVP))
        op = ctx.enter_context(tc.tile_pool(name="op", bufs=3))
        sm = ctx.enter_context(tc.tile_pool(name="sm", bufs=2))
        ps_sc = ctx.enter_context(tc.tile_pool(name="ps_sc", bufs=2, space="PSUM"))
        ps_at = ctx.enter_context(tc.tile_pool(name="ps_at", bufs=2, space="PSUM"))
        ps_o = ctx.enter_context(tc.tile_pool(name="ps_o", bufs=4, space="PSUM"))

        ident = singles.tile([_C, _C], f32)
        make_identity(nc, ident)

        def emit_qk_loads(b):
            # 1 MiB piece loads (separate tiles for fine-grained deps),
            # q/k swapped across the two rings per piece so each batch's
            # q+k pieces arrive together and the first matmuls can start
            # as early as possible
            qs, ks = [], []
            for pc in range(_NP):
                sl = slice(pc * _PIECE, (pc + 1) * _PIECE)
                qt = qp.tile([_H, _PIECE, _C], fp16, tag="qt")
                kt = kp.tile([_H, _PIECE, _C], fp16, tag="kt")
                qe, ke = (nc.scalar, nc.sync) if pc % 2 == 0 else (nc.sync, nc.scalar)
                qe.dma_start(out=qt, in_=q_ext[b, :, sl, :])
                ke.dma_start(out=kt, in_=k_ext[b, :, sl, :])
                qs.append(qt)
                ks.append(kt)
            return qs, ks

        def emit_vt_loads(b):
            vts = []
            for pc in range(_NVP):
                jsl = slice(pc * _VPIECE, (pc + 1) * _VPIECE)
                vtt = vtp.tile([2 * _C, _VPIECE, _H], fp16, tag="vtt")
                eng = nc.scalar if pc % 2 == 0 else nc.sync
                eng.dma_start(out=vtt, in_=vt_ext[b, :, jsl, :])
                vts.append(vtt)
            return vts

        def emit_phase_a(qs, ks):
            ppp = _PIECE // 2  # pairs per piece
            blocks = ps_sc.tile([2 * _C, 2, _C], f32, tag="blocks")
            for j in range(_PAIRS_TOT):
                pc, ji = divmod(j, ppp)
                nc.tensor.matmul(
                    blocks,
                    lhsT=qs[pc][:, 2 * ji : 2 * ji + 2, :],
                    rhs=ks[pc][:, 2 * ji : 2 * ji + 2, :],
                    start=(j == 0),
                    stop=(j == _PAIRS_TOT - 1),
                )
            return blocks

        def emit_softmax(blocks):
            # scores = even-w diag block + odd-w diag block
            s0 = sm.tile([_C, _C], f32, tag="s0")
            nc.vector.tensor_copy(out=s0, in_=blocks[0:_C, 0, :])
            scores = sm.tile([_C, _C], f32, tag="scores")
            nc.vector.tensor_tensor(
                out=scores,
                in0=blocks[_C : 2 * _C, 1, :],
                in1=s0,
                op=mybir.AluOpType.add,
            )
            negmax = sm.tile([_C, 1], f32, tag="negmax")
            nc.vector.tensor_reduce(
                out=negmax,
                in_=scores,
                axis=mybir.AxisListType.X,
                op=mybir.AluOpType.max,
                negate=True,
            )
            e = sm.tile([_C, _C], f32, tag="e")
            ssum = sm.tile([_C, 1], f32, tag="ssum")
            nc.scalar.activation(
                out=e,
                in_=scores,
                func=mybir.ActivationFunctionType.Exp,
                bias=negmax,
                scale=1.0,
                accum_out=ssum,
            )
            rsum = sm.tile([_C, 1], f32, tag="rsum")
            nc.vector.reciprocal(out=rsum, in_=ssum)
            attn = sm.tile([_C, _C], f32, tag="attn")
            nc.vector.tensor_scalar_mul(attn, e, rsum)

            attnT_ps = ps_at.tile([_C, _C], f32, tag="attnT_ps")
            nc.tensor.transpose(attnT_ps, attn, ident)
            bd = sm.tile([2 * _C, 2, _C], fp16, tag="bd")
            nc.vector.memset(bd, 0.0)
            nc.vector.tensor_copy(out=bd[0:_C, 0, :], in_=attnT_ps)
            nc.vector.tensor_copy(out=bd[_C : 2 * _C, 1, :], in_=attnT_ps)
            return bd

        def emit_phase_c(b, vts, bd, nstores):
            # SWDGE store first-byte latency is ~6 us from issue, so
            # the FIRST store's data readiness sets the store-stream
            # start: batch 0 stores in two 1 MiB halves (first issues
            # ~3 us earlier, at half-done), batch 1 as one 2 MiB store
            # (its data lands while earlier stores drain). Half-batch
            # stores use 8 KiB DRAM segments - same descriptor shape
            # as the q/k load pieces that sustain full rate.
            wsc = _W // nstores
            for sc in range(nstores):
                otile = op.tile([_H, wsc, _C], fp16, tag="otile")
                for wg in range(0, wsc // 2, 4):  # 4 pairs per PSUM bank
                    o_ps = ps_o.tile([_H, 8, _C], f32, tag="o_ps")
                    for half in range(4):
                        j = sc * (wsc // 2) + wg + half
                        pc, ji = divmod(j, _VPIECE)
                        nc.tensor.matmul(
                            o_ps[:, 2 * half : 2 * half + 2, :],
                            lhsT=vts[pc][:, ji, :],
                            rhs=bd,
                            start=True,
                            stop=True,
                        )
                    if (wg // 4) % 2 == 0:
                        nc.vector.tensor_copy(
                            out=otile[:, 2 * wg : 2 * wg + 8, :], in_=o_ps
                        )
                    else:
                        nc.scalar.activation(
                            out=otile[:, 2 * wg : 2 * wg + 8, :],
                            in_=o_ps,
                            func=mybir.ActivationFunctionType.Copy,
                        )
                wsl = slice(sc * wsc, (sc + 1) * wsc)
                nc.gpsimd.dma_start(out=o_ext[b, :, wsl, :], in_=otile)

        # all loads first (nothing compute-gated ahead of any DMA), in
        # consumption order: q0k0 -> vt0 -> q1k1 -> vt1. PC0 is emitted
        # before PA1 so the PE fills the window spent waiting for the
        # batch-1 q/k bytes with batch-0 phase-C work, moving PC0 and
        # its stores off the kernel tail.
        # loads in arrival order q0k0, q1k1, vt0, vt1 (4 DMAs per ring);
        # compute PA0, PA1 as their bytes land, then PC0, PC1 paced by
        # the trailing vt pieces, stores alternating across both rings
        qs0, ks0 = emit_qk_loads(0)
        qs1, ks1 = emit_qk_loads(1)
        vts0 = emit_vt_loads(0)
        vts1 = emit_vt_loads(1)
        blocks0 = emit_phase_a(qs0, ks0)
        bd0 = emit_softmax(blocks0)
        blocks1 = emit_phase_a(qs1, ks1)
        bd1 = emit_softmax(blocks1)
        emit_phase_c(0, vts0, bd0, 2)
        emit_phase_c(1, vts1, bd1, 1)

    nc.finalize()
    return nc


def _get_nc():
    if "nc" not in _cache:
        _cache["nc"] = _build_nc()
    return _cache["nc"]


def _prep_inputs(q, k, v):
    """Host-side layout prep: fp16 casts, V transposed per w-pair."""
    q16 = q.astype(np.float16)
    k16 = k.astype(np.float16)
    vb = v.astype(np.float16)  # [B, H, W, C]
    # vt[b, (dw c), j, h] = v[b, h, 2j+dw, c]
    x = vb.transpose(0, 2, 3, 1)  # [B, W, C, H]
    x = x.reshape(_B, _W // 2, 2, _C, _H)  # [B, j, dw, C, H]
    vt = np.ascontiguousarray(x.transpose(0, 2, 3, 1, 4)).reshape(
        _B, 2 * _C, _W // 2, _H
    )
    return q16, k16, vt


def run(inputs, trace=False):
    """Run the SPMD kernel. Returns (full_output, BassKernelResults)."""
    from concourse.bass_utils import run_bass_kernel_spmd

    q = np.asarray(inputs["query"], dtype=np.float32)
    k = np.asarray(inputs["keys"], dtype=np.float32)
    v = np.asarray(inputs["values"], dtype=np.float32)
    assert q.shape == (_B, _H, _W, _C), q.shape

    q16, k16, vt = _prep_inputs(q, k, v)

    nc = _get_nc()
    in_maps = []
    for i in range(_NCORES):
        sl = slice(i * _BPC, (i + 1) * _BPC)
        in_maps.append({"q16": q16[sl], "k16": k16[sl], "vt": vt[sl]})

    res = run_bass_kernel_spmd(
        nc, in_maps, core_ids=list(range(_NCORES)), trace=trace
    )
    out = np.concatenate(
        [res.results[i]["out"].astype(np.float32) for i in range(_NCORES)], axis=0
    )
    return out, res


def kernel(**inputs) -> np.ndarray:
    out, _ = run(inputs, trace=False)
    return out
